# revision 18
# baseline (speedup 1.0000x reference)
"""Bipartite neural message-passing layer on 8 Trainium2 NeuronCores.

Sharding: the dense (A=1024, B=1024) edge grid is split across 8 cores
along A (128 rows each).  The small MLP weights and nodes_b are
replicated; the per-B message sum is AllReduced across cores.

Device layout: all big matmuls contract over the feature dim, so
activations live transposed (features on partitions).  The host packs
each core's edge shard as tiles of shape (128, 512) where the partition
axis carries 4 blocks of 32 edge-features (4 different 512-edge chunks
stacked), and the edge-MLP weights are block-diagonalized to match, so
every matmul runs with K=128 (or 64) contraction instead of K=32/16.

Chunk numbering: a-row `a` (local) spans chunks 2a (b in [0,512)) and
2a+1 (b in [512,1024)).  Layer-2 tile G holds 8 chunks in row-blocks
k=0..7 with chunk = 8G + 2*(k%4) + (k//4): row-blocks 0-3 carry the
even (b-half-0) chunks and 4-7 the odd ones, which makes the msg_to_a
fold a single 32-aligned tensor_tensor and keeps per-b-half
accumulators in aligned 64-partition groups.

The a axis is processed in a residue-permuted order (column 32r+G maps
to a = 4G + r) so all small rearrangements are contiguous; the host
permutes nodes_a on the way in and un-permutes new_a on the way out.

All constants travel in ONE packed (128, W) input loaded by a single
DMA: the codegen's LDWEIGHTS struct only fits one sync-wait, so a
matmul whose lhsT and rhs wait on two different DMA semaphores fails to
compile ("Too many sync wait commands").  One constant-DMA means one
semaphore covers every weight.
"""

import sys

sys.path.insert(0, "/opt/trn_rl_repo")

import numpy as np

N_CORES = 8
N_A = 1024
N_B = 1024
NODE_DIM = 32
EDGE_DIM = 16
EDGE_FEAT = 2 * EDGE_DIM  # 32 channels of edge_embeds
A_SHARD = N_A // N_CORES  # 128 a-rows per core
CHUNK = 512               # edges per chunk (matmul free dim)
N_CHUNKS = A_SHARD * N_B // CHUNK      # 256 chunks per core
N_G1 = N_CHUNKS // 4                   # 64 layer-1 groups (4 chunks stacked)
N_G2 = N_CHUNKS // 8                   # 32 layer-2 groups (8 chunks stacked)

# (name, partitions, columns) of each slice in the packed const buffers.
# The R-buffer holds operands of float32r matmuls (the verifier requires
# f32r consumers to read from f32r-typed producers).
_CONSTR_SPEC = [
    ("w1e", 128, 64),     # block-diag x4 of We1[64:96]
    ("w1b4", 32, 64),     # tile(We1[32:64], (1,4))
    ("w2", 128, 128),     # block-diag x8 of We2
    ("b40", 32, CHUNK),   # nbT b-half 0
    ("b41", 32, CHUNK),   # nbT b-half 1
    ("naT", 32, A_SHARD), # nodes_a shard, transposed, residue-permuted
    ("nbT", 32, N_B),     # nodes_b transposed
    ("w1a", 32, 16),      # We1[0:32]
    ("sel4", 64, 16),     # tile(eye(16), (4,1))
    ("wn1a", 32, 32),     # Wn1[0:32]
    ("wn1m", 16, 32),     # Wn1[32:48]
    ("wn2", 32, 32),
]
CONSTR_W = sum(w for _, _, w in _CONSTR_SPEC)
_CONST_SPEC = [
    ("be1", 16, 1),
    ("be2", 128, 1),      # tile(be2, 8)
    ("bn1", 32, 1),
    ("bn2", 32, 1),
]
CONST_W = sum(w for _, _, w in _CONST_SPEC)

_BUILD_CACHE = {}


def _build_bass():
    """Trace the SPMD Bass/Tile program (identical on all 8 cores)."""
    import concourse.bacc as bacc
    import concourse.mybir as mybir
    import concourse.tile as tile

    f32 = mybir.dt.float32
    f32r = mybir.dt.float32r
    ADD = mybir.AluOpType.add
    MAX = mybir.AluOpType.max
    RELU = mybir.ActivationFunctionType.Relu

    # Bacc (not raw Bass): its lowering passes split multi-wait
    # instructions into event-semaphore chains, which the TRN2 ISA
    # requires (one sync-wait slot per instruction).
    nc = bacc.Bacc(
        "TRN2", target_bir_lowering=False, debug=False, num_devices=N_CORES
    )

    # ---- DRAM I/O (per core) ----
    edges = nc.dram_tensor(
        "edges_packed", [N_G1, 128, CHUNK], f32r, kind="ExternalInput"
    ).ap()
    consts_d = nc.dram_tensor(
        "consts", [128, CONST_W], f32, kind="ExternalInput"
    ).ap()
    constsr_d = nc.dram_tensor(
        "consts_r", [128, CONSTR_W], f32r, kind="ExternalInput"
    ).ap()
    out_packed = nc.dram_tensor(
        "out_packed", [N_G2, 128, CHUNK], f32, kind="ExternalOutput"
    ).ap()
    new_aT_d = nc.dram_tensor(
        "new_aTP", [32, A_SHARD], f32, kind="ExternalOutput"
    ).ap()
    new_bT_d = nc.dram_tensor("new_bT", [32, N_B], f32, kind="ExternalOutput").ap()

    with tile.TileContext(nc, num_cores=N_CORES) as tc:
        with (
            tc.tile_pool(name="const", bufs=1) as cpool,
            tc.tile_pool(name="epool", bufs=6) as epool,
            tc.tile_pool(name="hpool", bufs=3) as hpool,
            tc.tile_pool(name="npool", bufs=4) as npool,
            tc.tile_pool(name="dram", bufs=1, space="DRAM") as dpool,
        ):
            constr_sb = cpool.tile([128, CONSTR_W], f32r, name="constr_sb")
            nc.sync.dma_start(out=constr_sb[:], in_=constsr_d)
            const_sb = cpool.tile([128, CONST_W], f32, name="const_sb")
            nc.sync.dma_start(out=const_sb[:], in_=consts_d)
            C = {}
            off = 0
            for nm, p, w in _CONST_SPEC:
                C[nm] = const_sb[0:p, off : off + w]
                off += w
            off = 0
            for nm, p, w in _CONSTR_SPEC:
                C[nm] = constr_sb[0:p, off : off + w]
                off += w
            w1e, w1b4, w2, naT, nbT = (
                C["w1e"], C["w1b4"], C["w2"], C["naT"], C["nbT"]
            )
            b40, b41 = C["b40"], C["b41"]
            w1a, sel4, be1, be2 = C["w1a"], C["sel4"], C["be1"], C["be2"]
            wn1a, wn1m, wn2, bn1, bn2 = (
                C["wn1a"], C["wn1m"], C["wn2"], C["bn1"], C["bn2"]
            )

            R = cpool.tile([128, N_G2], f32, name="Racc")
            msgB = cpool.tile([128, CHUNK], f32, name="msgB")
            # row-blocks k and k+4 of an L2 tile carry the same a-values,
            # so one 64-partition bias-column tile serves both psum halves
            ca_cols = cpool.tile([64, N_G2], f32, name="ca_cols")
            ca_res = cpool.tile([16, A_SHARD], f32, name="ca_res")
            PB4A = cpool.tile([64, CHUNK], f32, name="PB4A")
            PB4B = cpool.tile([64, CHUNK], f32, name="PB4B")

            nc.vector.memset(msgB[:], 0.0)

            # ---- head: C_A bias columns + one-time B-term products ----
            # ca_res[m, 32r+G] = (We1_A.T @ nodes_a.T)[m, a=4G+r] + be1[m]
            # (permuted a-order); ca_cols[16k+m, G] = ca_res[m, 32*(k%4)+G].
            # PB4x[16k+m, j] = (We1_B.T @ nodes_b.T)[m, 512x+j], replicated
            # over the 4 stacked chunk blocks by the tiled weight matrix.
            with tc.tile_pool(name="ppx0", bufs=2, space="PSUM") as ppx0:
                ca_ps = ppx0.tile([16, A_SHARD], f32, name="ca_ps", tag="px")
                nc.tensor.matmul(ca_ps[:], w1a, naT, start=True, stop=True)
                nc.vector.tensor_scalar(
                    out=ca_res[:], in0=ca_ps[:], scalar1=be1, scalar2=None,
                    op0=ADD,
                )
                for r in range(4):
                    nc.sync.dma_start(
                        out=ca_cols[16 * r : 16 * r + 16, :],
                        in_=ca_res[:, 32 * r : 32 * r + 32],
                    )
                for pb_sb, b4x in ((PB4A, b40), (PB4B, b41)):
                    pb_ps = ppx0.tile([64, CHUNK], f32, name="pb_ps", tag="px")
                    nc.tensor.matmul(pb_ps[:], w1b4, b4x, start=True, stop=True)
                    nc.vector.tensor_scalar(
                        out=pb_sb[:], in0=pb_ps[:], scalar1=0.0, scalar2=None,
                        op0=ADD,
                    )

            # ---- main loop over layer-2 groups ----
            with (
                tc.tile_pool(name="pp1", bufs=2, space="PSUM") as pp1,
                tc.tile_pool(name="pp2", bufs=2, space="PSUM") as pp2,
            ):
                for G in range(N_G2):
                    # one 512 KiB load per G: both L1 tiles are adjacent
                    e01 = epool.tile([128, 2 * CHUNK], f32r, name="e01", tag="et")
                    nc.sync.dma_start(
                        out=e01[:],
                        in_=edges[2 * G : 2 * G + 2].rearrange("g p j -> p g j"),
                    )
                    e0 = e01[:, 0:CHUNK]
                    e1 = e01[:, CHUNK : 2 * CHUNK]

                    # float32r matmuls must write at psum base-partition 0,
                    # so the two stacked L1 halves use separate psum tiles
                    psA = pp1.tile([64, CHUNK], f32, name="psA", tag="psA")
                    psB = pp1.tile([64, CHUNK], f32, name="psB", tag="psB")
                    nc.tensor.matmul(psA[:], w1e, e0, start=True, stop=True)
                    nc.tensor.matmul(psB[:], w1e, e1, start=True, stop=True)

                    # h_pre = psum + CA bias column + B-term, then relu
                    hpA = hpool.tile([64, CHUNK], f32, name="hpA", tag="hpA")
                    hpB = hpool.tile([64, CHUNK], f32, name="hpB", tag="hpB")
                    nc.vector.scalar_tensor_tensor(
                        out=hpA[:], in0=psA[:], scalar=ca_cols[:, G : G + 1],
                        in1=PB4A[:], op0=ADD, op1=ADD,
                    )
                    nc.vector.scalar_tensor_tensor(
                        out=hpB[:], in0=psB[:], scalar=ca_cols[:, G : G + 1],
                        in1=PB4B[:], op0=ADD, op1=ADD,
                    )
                    h8 = hpool.tile([128, CHUNK], f32r, name="h8", tag="h8")
                    nc.scalar.activation(
                        out=h8[0:64, :], in_=hpA[:], func=RELU,
                    )
                    nc.gpsimd.tensor_scalar(
                        out=h8[64:128, :], in0=hpB[:], scalar1=0.0,
                        scalar2=None, op0=MAX,
                    )

                    ps2 = pp2.tile([128, CHUNK], f32, name="ps2", tag="ps2")
                    nc.tensor.matmul(ps2[:], w2, h8[:], start=True, stop=True)

                    nt = npool.tile([128, CHUNK], f32, name="nt", tag="nt")
                    nc.scalar.activation(
                        out=nt[:],
                        in_=ps2[:],
                        func=RELU,
                        bias=be2,
                        accum_out=R[:, G : G + 1],
                    )
                    nc.vector.tensor_tensor(
                        out=msgB[:], in0=msgB[:], in1=nt[:], op=ADD
                    )
                    nc.sync.dma_start(out=out_packed[G], in_=nt[:])

            # ---- tail: message folds, AllReduce, node MLPs ----
            with tc.tile_pool(name="ptail", bufs=1, space="PSUM") as pt:
                # msg_to_a: R row-blocks k and k+4 hold the two b-halves of
                # a = 4G + (k%4); fold, then regroup to the permuted-a
                # layout msg_aP[m, 32r+G] = msg_a[4G+r, m].
                R1 = cpool.tile([64, N_G2], f32, name="R1")
                nc.vector.tensor_scalar(
                    out=R1[:], in0=R[64:128, :], scalar1=0.0, scalar2=None,
                    op0=ADD,
                )
                Rp = cpool.tile([64, N_G2], f32, name="Rp")
                nc.vector.tensor_tensor(
                    out=Rp[:], in0=R[0:64, :], in1=R1[:], op=ADD
                )
                msg_aP = cpool.tile([16, A_SHARD], f32, name="msg_aP")
                for r in range(4):
                    nc.sync.dma_start(
                        out=msg_aP[:, 32 * r : 32 * r + 32],
                        in_=Rp[16 * r : 16 * r + 16, :],
                    )
                msg_aPr = cpool.tile([16, A_SHARD], f32r, name="msg_aPr")
                nc.vector.tensor_scalar(
                    out=msg_aPr[:], in0=msg_aP[:], scalar1=0.0, scalar2=None,
                    op0=ADD,
                )

                # msg_to_b (local partial): msgB rows [0:64] hold b-half 0
                # (4 a-residue blocks), rows [64:128] b-half 1.  Rebase the
                # halves into f32r tiles, then fold the 4 blocks with a 0/1
                # selection matmul: msg_b_half = sel4.T @ msgB_half.
                msgB0r = cpool.tile([64, CHUNK], f32r, name="msgB0r")
                msgB1r = cpool.tile([64, CHUNK], f32r, name="msgB1r")
                nc.vector.tensor_scalar(
                    out=msgB0r[:], in0=msgB[0:64, :], scalar1=0.0,
                    scalar2=None, op0=ADD,
                )
                nc.vector.tensor_scalar(
                    out=msgB1r[:], in0=msgB[64:128, :], scalar1=0.0,
                    scalar2=None, op0=ADD,
                )
                msg_bT = cpool.tile([16, N_B], f32r, name="msg_bT")
                for h, srcx in ((0, msgB0r), (1, msgB1r)):
                    ps_mb = pt.tile([16, CHUNK], f32, name=f"ps_mb{h}")
                    nc.tensor.matmul(
                        ps_mb[:], sel4, srcx[:], start=True, stop=True,
                    )
                    nc.vector.tensor_scalar(
                        out=msg_bT[:, CHUNK * h : CHUNK * (h + 1)],
                        in0=ps_mb[:],
                        scalar1=0.0,
                        scalar2=None,
                        op0=ADD,
                    )

                # AllReduce the per-B message partial across the 8 cores
                cc_in = dpool.tile([16, N_B], f32r, name="cc_in")
                cc_out = dpool.tile(
                    [16, N_B], f32r, name="cc_out", addr_space="Shared"
                )
                nc.gpsimd.dma_start(out=cc_in[:], in_=msg_bT[:])
                nc.gpsimd.collective_compute(
                    "AllReduce",
                    mybir.AluOpType.add,
                    replica_groups=[list(range(N_CORES))],
                    ins=[cc_in.opt()],
                    outs=[cc_out.opt()],
                )
                msg_bg = cpool.tile([16, N_B], f32r, name="msg_bg")
                nc.gpsimd.dma_start(out=msg_bg[:], in_=cc_out[:])

                # node MLP for the local a-shard (permuted a-order);
                # independent of the AllReduce, overlaps it
                ps_na = pt.tile([32, A_SHARD], f32, name="ps_na")
                nc.tensor.matmul(ps_na[:], wn1a, naT, start=True, stop=False)
                nc.tensor.matmul(
                    ps_na[:], wn1m, msg_aPr[:], start=False, stop=True
                )
                hA = cpool.tile([32, A_SHARD], f32r, name="hA")
                nc.vector.tensor_scalar(
                    out=hA[:], in0=ps_na[:], scalar1=bn1, scalar2=0.0,
                    op0=ADD, op1=MAX,
                )
                ps_na2 = pt.tile([32, A_SHARD], f32, name="ps_na2")
                nc.tensor.matmul(ps_na2[:], wn2, hA[:], start=True, stop=True)
                naO = cpool.tile([32, A_SHARD], f32, name="naO")
                nc.vector.tensor_scalar(
                    out=naO[:], in0=ps_na2[:], scalar1=bn2, scalar2=0.0,
                    op0=ADD, op1=MAX,
                )
                nc.sync.dma_start(out=new_aT_d, in_=naO[:])

                # node MLP for all of b (replicated on every core)
                hB = cpool.tile([32, N_B], f32r, name="hB")
                for h in range(2):
                    sl = slice(CHUNK * h, CHUNK * (h + 1))
                    ps_nb = pt.tile([32, CHUNK], f32, name=f"ps_nb{h}")
                    nc.tensor.matmul(
                        ps_nb[:], wn1a, nbT[:, sl], start=True, stop=False
                    )
                    nc.tensor.matmul(
                        ps_nb[:], wn1m, msg_bg[:, sl], start=False, stop=True
                    )
                    nc.vector.tensor_scalar(
                        out=hB[:, sl], in0=ps_nb[:], scalar1=bn1, scalar2=0.0,
                        op0=ADD, op1=MAX,
                    )
                nbO = cpool.tile([32, N_B], f32, name="nbO")
                for h in range(2):
                    sl = slice(CHUNK * h, CHUNK * (h + 1))
                    ps_nb2 = pt.tile([32, CHUNK], f32, name=f"ps_nb2{h}")
                    nc.tensor.matmul(
                        ps_nb2[:], wn2, hB[:, sl], start=True, stop=True
                    )
                    nc.vector.tensor_scalar(
                        out=nbO[:, sl], in0=ps_nb2[:], scalar1=bn2,
                        scalar2=0.0, op0=ADD, op1=MAX,
                    )
                nc.sync.dma_start(out=new_bT_d, in_=nbO[:])

    nc.compile()
    return nc


def get_bass():
    if "nc" not in _BUILD_CACHE:
        _BUILD_CACHE["nc"] = _build_bass()
    return _BUILD_CACHE["nc"]


def _block_diag(w, n):
    """(K, M) -> (n*K, n*M) block-diagonal."""
    K, M = w.shape
    out = np.zeros((n * K, n * M), dtype=w.dtype)
    for i in range(n):
        out[i * K : (i + 1) * K, i * M : (i + 1) * M] = w
    return out


def _pack(spec, width, vals):
    consts = np.zeros((128, width), np.float32)
    off = 0
    for nm, p, w in spec:
        v = vals[nm]
        assert v.shape == (p, w), (nm, v.shape, (p, w))
        consts[0:p, off : off + w] = v
        off += w
    return consts


def make_in_maps(edge_embeds, nodes_a_embeds, nodes_b_embeds,
                 We1, be1, We2, be2, Wn1, bn1, Wn2, bn2):
    f = np.float32
    edge_embeds = np.asarray(edge_embeds, f)
    nodes_a = np.asarray(nodes_a_embeds, f)
    nodes_b = np.asarray(nodes_b_embeds, f)
    We1 = np.asarray(We1, f); be1 = np.asarray(be1, f)
    We2 = np.asarray(We2, f); be2 = np.asarray(be2, f)
    Wn1 = np.asarray(Wn1, f); bn1 = np.asarray(bn1, f)
    Wn2 = np.asarray(Wn2, f); bn2 = np.asarray(bn2, f)

    nbT = np.ascontiguousarray(nodes_b.T)                            # (32, 1024)
    vals = {
        "w1e": _block_diag(We1[64:96], 4),
        "w1b4": np.tile(We1[32:64], (1, 4)),
        "w2": _block_diag(We2, 8),
        "b40": nbT[:, :CHUNK].copy(),
        "b41": nbT[:, CHUNK:].copy(),
        "nbT": nbT,
        "w1a": We1[0:32].copy(),
        "sel4": np.tile(np.eye(16, dtype=f), (4, 1)),
        "be1": be1.reshape(16, 1),
        "be2": np.tile(be2, 8).reshape(128, 1),
        "wn1a": Wn1[0:32].copy(),
        "wn1m": Wn1[32:48].copy(),
        "wn2": Wn2.copy(),
        "bn1": bn1.reshape(32, 1),
        "bn2": bn2.reshape(32, 1),
    }

    in_maps = []
    for i in range(N_CORES):
        shard = edge_embeds[i * A_SHARD : (i + 1) * A_SHARD]   # (128, 1024, 32)
        F = shard.reshape(A_SHARD * N_B, EDGE_FEAT)            # (131072, 32)
        # packed[2G+p, 32k+c, j] = F[(8G + 2k + p)*512 + j, c]
        packed = np.ascontiguousarray(
            F.reshape(N_G2, 4, 2, CHUNK, EDGE_FEAT).transpose(0, 2, 1, 4, 3)
        ).reshape(N_G1, 128, CHUNK)
        na = nodes_a[i * A_SHARD : (i + 1) * A_SHARD]          # (128, 32)
        # permuted a-order: column 32r+G <- a = 4G+r
        vals["naT"] = (
            na.reshape(N_G2, 4, NODE_DIM).transpose(1, 0, 2)
            .reshape(A_SHARD, NODE_DIM).T.copy()
        )                                                      # (32, 128)
        in_maps.append(
            {
                "edges_packed": packed,
                "consts": _pack(_CONST_SPEC, CONST_W, vals),
                "consts_r": _pack(_CONSTR_SPEC, CONSTR_W, vals),
            }
        )
    return in_maps


def unpack_outputs(results):
    """results: list (per core) of dicts with out_packed/new_aTP/new_bT."""
    new_edges = np.empty((N_A, N_B, EDGE_DIM), np.float32)
    new_a = np.empty((N_A, NODE_DIM), np.float32)
    for i, r in enumerate(results):
        O = np.asarray(r["out_packed"])                # (32, 128, 512)
        # O[G, 16k+m, j]: chunk = 8G + 2*(k%4) + k//4,
        # a = 4G + k%4 (local), b = 512*(k//4) + j, channel m.
        shard = (
            O.reshape(N_G2, 2, 4, EDGE_DIM, CHUNK)     # [G, p, r, m, j]
            .transpose(0, 2, 1, 4, 3)                  # [G, r, p, j, m]
            .reshape(A_SHARD, N_B, EDGE_DIM)
        )
        new_edges[i * A_SHARD : (i + 1) * A_SHARD] = shard
        # new_aTP column 32r+G -> a = 4G+r
        naP = np.asarray(r["new_aTP"]).T               # (128, 32) permuted rows
        na = naP.reshape(4, N_G2, NODE_DIM).transpose(1, 0, 2).reshape(
            A_SHARD, NODE_DIM
        )
        new_a[i * A_SHARD : (i + 1) * A_SHARD] = na
    new_b = np.ascontiguousarray(np.asarray(results[0]["new_bT"]).T)
    return new_edges, new_a, new_b


def kernel(edge_embeds, nodes_a_embeds, nodes_b_embeds,
           We1, be1, We2, be2, Wn1, bn1, Wn2, bn2, _run_kwargs=None):
    from concourse.bass_utils import run_bass_kernel_spmd

    nc = get_bass()
    in_maps = make_in_maps(
        edge_embeds, nodes_a_embeds, nodes_b_embeds,
        We1, be1, We2, be2, Wn1, bn1, Wn2, bn2,
    )
    kw = _run_kwargs or {}
    out = run_bass_kernel_spmd(nc, in_maps, core_ids=list(range(N_CORES)), **kw)
    results = out.results
    kernel.last_run = out
    return unpack_outputs(results)


# revision 20
# speedup vs baseline: 2.2701x; 2.2701x over previous
"""Bipartite neural message-passing layer on 8 Trainium2 NeuronCores.

Sharding: the dense (A=1024, B=1024) edge grid is split across 8 cores
along A (128 rows each).  The small MLP weights and nodes_b are
replicated; the per-B message sum is AllReduced across cores.

Device layout: all big matmuls contract over the feature dim, so
activations live transposed (features on partitions).  The host packs
each core's edge shard as tiles of shape (128, 512) where the partition
axis carries 4 blocks of 32 edge-features (4 different 512-edge chunks
stacked), and the edge-MLP weights are block-diagonalized to match, so
every matmul runs with K=128 (or 64) contraction instead of K=32/16.

Chunk numbering: a-row `a` (local) spans chunks 2a (b in [0,512)) and
2a+1 (b in [512,1024)).  Layer-2 tile G holds 8 chunks in row-blocks
k=0..7 with chunk = 8G + 2*(k%4) + (k//4): row-blocks 0-3 carry the
even (b-half-0) chunks and 4-7 the odd ones, which makes the msg_to_a
fold a single 32-aligned tensor_tensor and keeps per-b-half
accumulators in aligned 64-partition groups.

The a axis is processed in a residue-permuted order (column 32r+G maps
to a = 4G + r) so all small rearrangements are contiguous; the host
permutes nodes_a on the way in and un-permutes new_a on the way out.

All constants travel in ONE packed (128, W) input loaded by a single
DMA: the codegen's LDWEIGHTS struct only fits one sync-wait, so a
matmul whose lhsT and rhs wait on two different DMA semaphores fails to
compile ("Too many sync wait commands").  One constant-DMA means one
semaphore covers every weight.
"""

import sys

sys.path.insert(0, "/opt/trn_rl_repo")

import numpy as np

N_CORES = 8
N_A = 1024
N_B = 1024
NODE_DIM = 32
EDGE_DIM = 16
EDGE_FEAT = 2 * EDGE_DIM  # 32 channels of edge_embeds
A_SHARD = N_A // N_CORES  # 128 a-rows per core
CHUNK = 512               # edges per chunk (matmul free dim)
N_CHUNKS = A_SHARD * N_B // CHUNK      # 256 chunks per core
N_G1 = N_CHUNKS // 4                   # 64 layer-1 groups (4 chunks stacked)
N_G2 = N_CHUNKS // 8                   # 32 layer-2 groups (8 chunks stacked)

# (name, partitions, columns) of each slice in the packed const buffers.
# The R-buffer holds operands of float32r matmuls (the verifier requires
# f32r consumers to read from f32r-typed producers).
_CONSTR_SPEC = [
    ("w1e", 128, 64),     # block-diag x4 of We1[64:96]
    ("w1b4", 32, 64),     # tile(We1[32:64], (1,4))
    ("w2", 128, 128),     # block-diag x8 of We2
    ("b40", 32, CHUNK),   # nbT b-half 0
    ("b41", 32, CHUNK),   # nbT b-half 1
    ("naT", 32, A_SHARD), # nodes_a shard, transposed, residue-permuted
    ("nbT", 32, N_B),     # nodes_b transposed
    ("w1a", 32, 16),      # We1[0:32]
    ("sel8", 128, 64),    # per-b-half 0/1 channel-fold matrix
    ("wn1a", 32, 32),     # Wn1[0:32]
    ("wn1m", 16, 32),     # Wn1[32:48]
    ("wn2", 32, 32),
]
CONSTR_W = sum(w for _, _, w in _CONSTR_SPEC)
_CONST_SPEC = [
    ("be1", 16, 1),
    ("be2", 128, 1),      # tile(be2, 8)
    ("bn1", 32, 1),
    ("bn2", 32, 1),
]
CONST_W = sum(w for _, _, w in _CONST_SPEC)

_BUILD_CACHE = {}


def _build_bass():
    """Trace the SPMD Bass/Tile program (identical on all 8 cores)."""
    import concourse.bacc as bacc
    import concourse.mybir as mybir
    import concourse.tile as tile

    f32 = mybir.dt.float32
    f32r = mybir.dt.float32r
    ADD = mybir.AluOpType.add
    MAX = mybir.AluOpType.max
    RELU = mybir.ActivationFunctionType.Relu

    # Bacc (not raw Bass): its lowering passes split multi-wait
    # instructions into event-semaphore chains, which the TRN2 ISA
    # requires (one sync-wait slot per instruction).
    nc = bacc.Bacc(
        "TRN2", target_bir_lowering=False, debug=False, num_devices=N_CORES
    )

    # ---- DRAM I/O (per core) ----
    edges = nc.dram_tensor(
        "edges_packed", [N_G1, 128, CHUNK], f32r, kind="ExternalInput"
    ).ap()
    consts_d = nc.dram_tensor(
        "consts", [128, CONST_W], f32, kind="ExternalInput"
    ).ap()
    constsr_d = nc.dram_tensor(
        "consts_r", [128, CONSTR_W], f32r, kind="ExternalInput"
    ).ap()
    out_packed = nc.dram_tensor(
        "out_packed", [N_G2, 128, CHUNK], f32r, kind="ExternalOutput"
    ).ap()
    new_aT_d = nc.dram_tensor(
        "new_aTP", [32, A_SHARD], f32, kind="ExternalOutput"
    ).ap()
    new_bT_d = nc.dram_tensor("new_bT", [32, N_B], f32, kind="ExternalOutput").ap()

    with tile.TileContext(nc, num_cores=N_CORES) as tc:
        with (
            tc.tile_pool(name="const", bufs=1) as cpool,
            tc.tile_pool(name="epool", bufs=6) as epool,
            tc.tile_pool(name="hpool", bufs=3) as hpool,
            tc.tile_pool(name="npool", bufs=4) as npool,
            tc.tile_pool(name="dram", bufs=1, space="DRAM") as dpool,
        ):
            constr_sb = cpool.tile([128, CONSTR_W], f32r, name="constr_sb")
            nc.sync.dma_start(out=constr_sb[:], in_=constsr_d)
            const_sb = cpool.tile([128, CONST_W], f32, name="const_sb")
            nc.sync.dma_start(out=const_sb[:], in_=consts_d)
            C = {}
            off = 0
            for nm, p, w in _CONST_SPEC:
                C[nm] = const_sb[0:p, off : off + w]
                off += w
            off = 0
            for nm, p, w in _CONSTR_SPEC:
                C[nm] = constr_sb[0:p, off : off + w]
                off += w
            w1e, w1b4, w2, naT, nbT = (
                C["w1e"], C["w1b4"], C["w2"], C["naT"], C["nbT"]
            )
            b40, b41 = C["b40"], C["b41"]
            w1a, sel8, be1, be2 = C["w1a"], C["sel8"], C["be1"], C["be2"]
            wn1a, wn1m, wn2, bn1, bn2 = (
                C["wn1a"], C["wn1m"], C["wn2"], C["bn1"], C["bn2"]
            )

            R = cpool.tile([128, N_G2], f32, name="Racc")
            # row-blocks k and k+4 of an L2 tile carry the same a-values,
            # so one 64-partition bias-column tile serves both psum halves
            ca_cols = cpool.tile([64, N_G2], f32, name="ca_cols")
            ca_res = cpool.tile([16, A_SHARD], f32, name="ca_res")
            PB4A = cpool.tile([64, CHUNK], f32, name="PB4A")
            PB4B = cpool.tile([64, CHUNK], f32, name="PB4B")

            # ---- head: C_A bias columns + one-time B-term products ----
            # ca_res[m, 32r+G] = (We1_A.T @ nodes_a.T)[m, a=4G+r] + be1[m]
            # (permuted a-order); ca_cols[16k+m, G] = ca_res[m, 32*(k%4)+G].
            # PB4x[16k+m, j] = (We1_B.T @ nodes_b.T)[m, 512x+j], replicated
            # over the 4 stacked chunk blocks by the tiled weight matrix.
            with tc.tile_pool(name="ppx0", bufs=2, space="PSUM") as ppx0:
                ca_ps = ppx0.tile([16, A_SHARD], f32, name="ca_ps", tag="px")
                nc.tensor.matmul(ca_ps[:], w1a, naT, start=True, stop=True)
                nc.vector.tensor_scalar(
                    out=ca_res[:], in0=ca_ps[:], scalar1=be1, scalar2=None,
                    op0=ADD,
                )
                for r in range(4):
                    nc.sync.dma_start(
                        out=ca_cols[16 * r : 16 * r + 16, :],
                        in_=ca_res[:, 32 * r : 32 * r + 32],
                    )
                for pb_sb, b4x in ((PB4A, b40), (PB4B, b41)):
                    pb_ps = ppx0.tile([64, CHUNK], f32, name="pb_ps", tag="px")
                    nc.tensor.matmul(pb_ps[:], w1b4, b4x, start=True, stop=True)
                    nc.vector.tensor_scalar(
                        out=pb_sb[:], in0=pb_ps[:], scalar1=0.0, scalar2=None,
                        op0=ADD,
                    )

            # ---- main loop over layer-2 groups ----
            with (
                tc.tile_pool(name="pp1", bufs=2, space="PSUM") as pp1,
                tc.tile_pool(name="pp2", bufs=2, space="PSUM") as pp2,
                tc.tile_pool(name="pmsg", bufs=1, space="PSUM") as pmsg,
            ):
                # per-B message accumulator, summed across all 32 groups in
                # PSUM by the per-G selection matmul: rows [0:16] b-half 0,
                # rows [32:48] b-half 1
                msgb_ps = pmsg.tile([64, CHUNK], f32, name="msgb_ps")
                for G in range(N_G2):
                    # one 512 KiB load per G: both L1 tiles are adjacent
                    e01 = epool.tile([128, 2 * CHUNK], f32r, name="e01", tag="et")
                    nc.sync.dma_start(
                        out=e01[:],
                        in_=edges[2 * G : 2 * G + 2].rearrange("g p j -> p g j"),
                    )
                    e0 = e01[:, 0:CHUNK]
                    e1 = e01[:, CHUNK : 2 * CHUNK]

                    # float32r matmuls must write at psum base-partition 0,
                    # so the two stacked L1 halves use separate psum tiles
                    psA = pp1.tile([64, CHUNK], f32, name="psA", tag="psA")
                    psB = pp1.tile([64, CHUNK], f32, name="psB", tag="psB")
                    nc.tensor.matmul(psA[:], w1e, e0, start=True, stop=True)
                    nc.tensor.matmul(psB[:], w1e, e1, start=True, stop=True)

                    # h_pre = psum + CA bias column + B-term, then relu
                    hpA = hpool.tile([64, CHUNK], f32, name="hpA", tag="hpA")
                    hpB = hpool.tile([64, CHUNK], f32, name="hpB", tag="hpB")
                    nc.vector.scalar_tensor_tensor(
                        out=hpA[:], in0=psA[:], scalar=ca_cols[:, G : G + 1],
                        in1=PB4A[:], op0=ADD, op1=ADD,
                    )
                    nc.vector.scalar_tensor_tensor(
                        out=hpB[:], in0=psB[:], scalar=ca_cols[:, G : G + 1],
                        in1=PB4B[:], op0=ADD, op1=ADD,
                    )
                    h8 = hpool.tile([128, CHUNK], f32r, name="h8", tag="h8")
                    nc.scalar.activation(
                        out=h8[0:64, :], in_=hpA[:], func=RELU,
                    )
                    nc.vector.tensor_scalar(
                        out=h8[64:128, :], in0=hpB[:], scalar1=0.0,
                        scalar2=None, op0=MAX,
                    )

                    ps2 = pp2.tile([128, CHUNK], f32, name="ps2", tag="ps2")
                    nc.tensor.matmul(ps2[:], w2, h8[:], start=True, stop=True)

                    nt = npool.tile([128, CHUNK], f32r, name="nt", tag="nt")
                    nc.scalar.activation(
                        out=nt[:],
                        in_=ps2[:],
                        func=RELU,
                        bias=be2,
                        accum_out=R[:, G : G + 1],
                    )
                    nc.tensor.matmul(
                        msgb_ps[:], sel8, nt[:],
                        start=(G == 0), stop=(G == N_G2 - 1),
                        skip_group_check=True,
                    )
                    nc.sync.dma_start(out=out_packed[G], in_=nt[:])

                # extract the two accumulated b-half sums while pmsg is live
                msg_bT = cpool.tile([16, N_B], f32r, name="msg_bT")
                nc.vector.tensor_scalar(
                    out=msg_bT[:, 0:CHUNK], in0=msgb_ps[0:16, :],
                    scalar1=0.0, scalar2=None, op0=ADD,
                )
                nc.vector.tensor_scalar(
                    out=msg_bT[:, CHUNK : N_B], in0=msgb_ps[32:48, :],
                    scalar1=0.0, scalar2=None, op0=ADD,
                )

            # ---- tail: message folds, AllReduce, node MLPs ----
            with tc.tile_pool(name="ptail", bufs=1, space="PSUM") as pt:
                # msg_to_a: R row-blocks k and k+4 hold the two b-halves of
                # a = 4G + (k%4); fold, then regroup to the permuted-a
                # layout msg_aP[m, 32r+G] = msg_a[4G+r, m].
                R1 = cpool.tile([64, N_G2], f32, name="R1")
                nc.vector.tensor_scalar(
                    out=R1[:], in0=R[64:128, :], scalar1=0.0, scalar2=None,
                    op0=ADD,
                )
                Rp = cpool.tile([64, N_G2], f32, name="Rp")
                nc.vector.tensor_tensor(
                    out=Rp[:], in0=R[0:64, :], in1=R1[:], op=ADD
                )
                msg_aP = cpool.tile([16, A_SHARD], f32, name="msg_aP")
                for r in range(4):
                    nc.sync.dma_start(
                        out=msg_aP[:, 32 * r : 32 * r + 32],
                        in_=Rp[16 * r : 16 * r + 16, :],
                    )
                msg_aPr = cpool.tile([16, A_SHARD], f32r, name="msg_aPr")
                nc.vector.tensor_scalar(
                    out=msg_aPr[:], in0=msg_aP[:], scalar1=0.0, scalar2=None,
                    op0=ADD,
                )

                # AllReduce the per-B message partial across the 8 cores
                cc_in = dpool.tile([16, N_B], f32r, name="cc_in")
                cc_out = dpool.tile(
                    [16, N_B], f32r, name="cc_out", addr_space="Shared"
                )
                nc.gpsimd.dma_start(out=cc_in[:], in_=msg_bT[:])
                nc.gpsimd.collective_compute(
                    "AllReduce",
                    mybir.AluOpType.add,
                    replica_groups=[list(range(N_CORES))],
                    ins=[cc_in.opt()],
                    outs=[cc_out.opt()],
                )
                msg_bg = cpool.tile([16, N_B], f32r, name="msg_bg")
                nc.gpsimd.dma_start(out=msg_bg[:], in_=cc_out[:])

                # node MLP for the local a-shard (permuted a-order);
                # independent of the AllReduce, overlaps it
                ps_na = pt.tile([32, A_SHARD], f32, name="ps_na")
                nc.tensor.matmul(ps_na[:], wn1a, naT, start=True, stop=False)
                nc.tensor.matmul(
                    ps_na[:], wn1m, msg_aPr[:], start=False, stop=True
                )
                hA = cpool.tile([32, A_SHARD], f32r, name="hA")
                nc.vector.tensor_scalar(
                    out=hA[:], in0=ps_na[:], scalar1=bn1, scalar2=0.0,
                    op0=ADD, op1=MAX,
                )
                ps_na2 = pt.tile([32, A_SHARD], f32, name="ps_na2")
                nc.tensor.matmul(ps_na2[:], wn2, hA[:], start=True, stop=True)
                naO = cpool.tile([32, A_SHARD], f32, name="naO")
                nc.vector.tensor_scalar(
                    out=naO[:], in0=ps_na2[:], scalar1=bn2, scalar2=0.0,
                    op0=ADD, op1=MAX,
                )
                nc.sync.dma_start(out=new_aT_d, in_=naO[:])

                # node MLP for all of b (replicated on every core)
                hB = cpool.tile([32, N_B], f32r, name="hB")
                for h in range(2):
                    sl = slice(CHUNK * h, CHUNK * (h + 1))
                    ps_nb = pt.tile([32, CHUNK], f32, name=f"ps_nb{h}")
                    nc.tensor.matmul(
                        ps_nb[:], wn1a, nbT[:, sl], start=True, stop=False
                    )
                    nc.tensor.matmul(
                        ps_nb[:], wn1m, msg_bg[:, sl], start=False, stop=True
                    )
                    nc.vector.tensor_scalar(
                        out=hB[:, sl], in0=ps_nb[:], scalar1=bn1, scalar2=0.0,
                        op0=ADD, op1=MAX,
                    )
                nbO = cpool.tile([32, N_B], f32, name="nbO")
                for h in range(2):
                    sl = slice(CHUNK * h, CHUNK * (h + 1))
                    ps_nb2 = pt.tile([32, CHUNK], f32, name=f"ps_nb2{h}")
                    nc.tensor.matmul(
                        ps_nb2[:], wn2, hB[:, sl], start=True, stop=True
                    )
                    nc.vector.tensor_scalar(
                        out=nbO[:, sl], in0=ps_nb2[:], scalar1=bn2,
                        scalar2=0.0, op0=ADD, op1=MAX,
                    )
                nc.sync.dma_start(out=new_bT_d, in_=nbO[:])

    nc.compile()
    return nc


def get_bass():
    if "nc" not in _BUILD_CACHE:
        _BUILD_CACHE["nc"] = _build_bass()
    return _BUILD_CACHE["nc"]


def _sel8():
    s = np.zeros((128, 64), np.float32)
    for k in range(8):
        h = k // 4
        s[16 * k : 16 * k + 16, 32 * h : 32 * h + 16] = np.eye(16, dtype=np.float32)
    return s


def _block_diag(w, n):
    """(K, M) -> (n*K, n*M) block-diagonal."""
    K, M = w.shape
    out = np.zeros((n * K, n * M), dtype=w.dtype)
    for i in range(n):
        out[i * K : (i + 1) * K, i * M : (i + 1) * M] = w
    return out


def _pack(spec, width, vals):
    consts = np.zeros((128, width), np.float32)
    off = 0
    for nm, p, w in spec:
        v = vals[nm]
        assert v.shape == (p, w), (nm, v.shape, (p, w))
        consts[0:p, off : off + w] = v
        off += w
    return consts


def make_in_maps(edge_embeds, nodes_a_embeds, nodes_b_embeds,
                 We1, be1, We2, be2, Wn1, bn1, Wn2, bn2):
    f = np.float32
    edge_embeds = np.asarray(edge_embeds, f)
    nodes_a = np.asarray(nodes_a_embeds, f)
    nodes_b = np.asarray(nodes_b_embeds, f)
    We1 = np.asarray(We1, f); be1 = np.asarray(be1, f)
    We2 = np.asarray(We2, f); be2 = np.asarray(be2, f)
    Wn1 = np.asarray(Wn1, f); bn1 = np.asarray(bn1, f)
    Wn2 = np.asarray(Wn2, f); bn2 = np.asarray(bn2, f)

    nbT = np.ascontiguousarray(nodes_b.T)                            # (32, 1024)
    vals = {
        "w1e": _block_diag(We1[64:96], 4),
        "w1b4": np.tile(We1[32:64], (1, 4)),
        "w2": _block_diag(We2, 8),
        "b40": nbT[:, :CHUNK].copy(),
        "b41": nbT[:, CHUNK:].copy(),
        "nbT": nbT,
        "w1a": We1[0:32].copy(),
        "sel8": _sel8(),
        "be1": be1.reshape(16, 1),
        "be2": np.tile(be2, 8).reshape(128, 1),
        "wn1a": Wn1[0:32].copy(),
        "wn1m": Wn1[32:48].copy(),
        "wn2": Wn2.copy(),
        "bn1": bn1.reshape(32, 1),
        "bn2": bn2.reshape(32, 1),
    }

    in_maps = []
    for i in range(N_CORES):
        shard = edge_embeds[i * A_SHARD : (i + 1) * A_SHARD]   # (128, 1024, 32)
        F = shard.reshape(A_SHARD * N_B, EDGE_FEAT)            # (131072, 32)
        # packed[2G+p, 32k+c, j] = F[(8G + 2k + p)*512 + j, c]
        packed = np.ascontiguousarray(
            F.reshape(N_G2, 4, 2, CHUNK, EDGE_FEAT).transpose(0, 2, 1, 4, 3)
        ).reshape(N_G1, 128, CHUNK)
        na = nodes_a[i * A_SHARD : (i + 1) * A_SHARD]          # (128, 32)
        # permuted a-order: column 32r+G <- a = 4G+r
        vals["naT"] = (
            na.reshape(N_G2, 4, NODE_DIM).transpose(1, 0, 2)
            .reshape(A_SHARD, NODE_DIM).T.copy()
        )                                                      # (32, 128)
        in_maps.append(
            {
                "edges_packed": packed,
                "consts": _pack(_CONST_SPEC, CONST_W, vals),
                "consts_r": _pack(_CONSTR_SPEC, CONSTR_W, vals),
            }
        )
    return in_maps


def unpack_outputs(results):
    """results: list (per core) of dicts with out_packed/new_aTP/new_bT."""
    new_edges = np.empty((N_A, N_B, EDGE_DIM), np.float32)
    new_a = np.empty((N_A, NODE_DIM), np.float32)
    for i, r in enumerate(results):
        O = np.asarray(r["out_packed"])                # (32, 128, 512)
        # O[G, 16k+m, j]: chunk = 8G + 2*(k%4) + k//4,
        # a = 4G + k%4 (local), b = 512*(k//4) + j, channel m.
        shard = (
            O.reshape(N_G2, 2, 4, EDGE_DIM, CHUNK)     # [G, p, r, m, j]
            .transpose(0, 2, 1, 4, 3)                  # [G, r, p, j, m]
            .reshape(A_SHARD, N_B, EDGE_DIM)
        )
        new_edges[i * A_SHARD : (i + 1) * A_SHARD] = shard
        # new_aTP column 32r+G -> a = 4G+r
        naP = np.asarray(r["new_aTP"]).T               # (128, 32) permuted rows
        na = naP.reshape(4, N_G2, NODE_DIM).transpose(1, 0, 2).reshape(
            A_SHARD, NODE_DIM
        )
        new_a[i * A_SHARD : (i + 1) * A_SHARD] = na
    new_b = np.ascontiguousarray(np.asarray(results[0]["new_bT"]).T)
    return new_edges, new_a, new_b


def kernel(edge_embeds, nodes_a_embeds, nodes_b_embeds,
           We1, be1, We2, be2, Wn1, bn1, Wn2, bn2, _run_kwargs=None):
    from concourse.bass_utils import run_bass_kernel_spmd

    nc = get_bass()
    in_maps = make_in_maps(
        edge_embeds, nodes_a_embeds, nodes_b_embeds,
        We1, be1, We2, be2, Wn1, bn1, Wn2, bn2,
    )
    kw = _run_kwargs or {}
    out = run_bass_kernel_spmd(nc, in_maps, core_ids=list(range(N_CORES)), **kw)
    results = out.results
    kernel.last_run = out
    return unpack_outputs(results)
